# revision 7
# baseline (speedup 1.0000x reference)
"""Trainium2 Bass kernel for nn_FCLSTM: embedding -> custom LSTM-ish recurrence -> select -> linear -> log_softmax.

Self-contained: hardcodes shapes. kernel(**inputs) takes full numpy inputs, returns [64, 2] fp32.

v2 redesign vs baseline:
  - State transform h' = 2h - 1 turns sigmoid(f)+tanh(g)*inp into tanh(f')+tanh(g')*inp2
    with all scales/biases folded into the weights host-side -> ONE tanh ACT op per
    psum half ([128,512]) instead of sigmoid+tanh pairs.
  - Bias matmuls issued as next-step PSUM group openers (fill the PE pipeline bubble).
  - h_new -> hT transposes moved off the PE onto the DMA XBAR transpose engine.
  - Per-chunk hT tiles so next-step matmuls wait only on their own chunk.
  - Contiguous embedding-table layout (one [128,512] DMA per vocab tile).
"""
import os
import numpy as np

import concourse.bacc as bacc
import concourse.bass as bass
import concourse.mybir as mybir
from concourse import library_config  # noqa: F401
from concourse.tile import TileContext
from concourse.masks import make_identity
from concourse.bass_utils import run_bass_kernel_spmd

VOCAB, EMBED, HIDDEN, NCLS = 32000, 512, 1024, 2
B, S = 64, 512
NCORES = 8
HC = HIDDEN // NCORES          # 128 per-core H slice for the U table
NVT = VOCAB // 128             # 250 vocab tiles
NEC = EMBED // 128             # 4 embed (contraction) chunks
NKC = HIDDEN // 128            # 8 hidden contraction chunks
TCH = S // 8                   # 64 steps per AllGather time-chunk
TOK = B * S                    # 32768 tokens
F16 = mybir.dt.float16
F32 = mybir.dt.float32
I32 = mybir.dt.int32
Tanh = mybir.ActivationFunctionType.Tanh

# Plan A: one tanh per [128,512] psum half; DVE mul reads T at partition base 64.
# Plan B fallback (DVE_SHIFT=False): two tanh ACT ops per half, both landing at base 0
# (partition-shifted ACT reads, baseline-proven).
DVE_SHIFT = False

_CACHE = {}


def _build(steps=S):
    nc = bacc.Bacc("TRN2", target_bir_lowering=False, debug=False, num_devices=NCORES)

    # ---------- inputs ----------
    embt = nc.dram_tensor("embt", [NVT * 128, EMBED], F16, kind="ExternalInput")
    wi = nc.dram_tensor("wi", [EMBED, HC], F16, kind="ExternalInput")
    bi = nc.dram_tensor("bi", [1, HC], F16, kind="ExternalInput")
    wf = nc.dram_tensor("wf", [HIDDEN, HIDDEN], F16, kind="ExternalInput")
    wh = nc.dram_tensor("wh", [HIDDEN, HIDDEN], F16, kind="ExternalInput")
    bf_r = nc.dram_tensor("bf_r", [1, HIDDEN], F16, kind="ExternalInput")
    bh_r = nc.dram_tensor("bh_r", [1, HIDDEN], F16, kind="ExternalInput")
    wo = nc.dram_tensor("wo", [HIDDEN, HIDDEN], F16, kind="ExternalInput")
    bo_r = nc.dram_tensor("bo_r", [1, HIDDEN], F16, kind="ExternalInput")
    wlin = nc.dram_tensor("wlin", [HIDDEN, NCLS], F16, kind="ExternalInput")
    idx = nc.dram_tensor("idx", [128, TOK // 128], I32, kind="ExternalInput")
    selidx = nc.dram_tensor("selidx", [128, 1], I32, kind="ExternalInput")
    out_ext = nc.dram_tensor("out", [B, NCLS], F32, kind="ExternalOutput")

    ntch = (steps + TCH - 1) // TCH  # number of time chunks actually used

    with TileContext(nc) as tc:
        with (
            tc.tile_pool(name="dram", bufs=1, space="DRAM") as dram,
            tc.tile_pool(name="const", bufs=1) as cst,
            tc.tile_pool(name="w", bufs=1) as wpool,
            tc.tile_pool(name="uph", bufs=4) as uph,
            tc.tile_pool(name="upsum", bufs=2, space="PSUM") as upsum,
            tc.tile_pool(name="rec", bufs=2) as rec,
            tc.tile_pool(name="inp", bufs=3) as inpool,
            tc.tile_pool(name="gpsum", bufs=2, space="PSUM") as gpsum,
            tc.tile_pool(name="tpsum", bufs=1, space="PSUM") as tpsum,
        ):
            # ---------- DRAM scratch ----------
            u_dram = dram.tile([VOCAB, HC], F16)
            agin = [dram.tile([B * TCH, HC], F16, name=f"agin{j}") for j in range(ntch)]
            gath = [dram.tile([NCORES * B * TCH, HC], F16, name=f"gath{j}", addr_space="Shared") for j in range(ntch)]
            ring = dram.tile([TOK, HIDDEN], F16)

            # ---------- constants / weights to SBUF ----------
            ones64 = cst.tile([1, 64], F16, tag="ones64")
            nc.vector.memset(ones64[:], 1.0)
            ones128 = cst.tile([1, 128], F16, tag="ones128")
            nc.vector.memset(ones128[:], 1.0)
            ident = cst.tile([64, 64], F16, tag="ident")
            make_identity(nc, ident[:])

            wi_sb = cst.tile([128, NEC * HC], F16, tag="wi")
            for e in range(NEC):
                nc.sync.dma_start(out=wi_sb[:, e * HC:(e + 1) * HC],
                                  in_=wi[e * 128:(e + 1) * 128, :])
            bi_sb = cst.tile([1, HC], F16, tag="bi")
            nc.sync.dma_start(out=bi_sb[:], in_=bi[:])
            bf_sb = cst.tile([1, HIDDEN], F16, tag="bf")
            nc.sync.dma_start(out=bf_sb[:], in_=bf_r[:])
            bh_sb = cst.tile([1, HIDDEN], F16, tag="bh")
            nc.sync.dma_start(out=bh_sb[:], in_=bh_r[:])
            bo_sb = cst.tile([1, HIDDEN], F16, tag="bo")
            nc.sync.dma_start(out=bo_sb[:], in_=bo_r[:])

            wf_sb = wpool.tile([128, NKC * HIDDEN], F16, tag="wf")
            wh_sb = wpool.tile([128, NKC * HIDDEN], F16, tag="wh")
            for k in range(NKC):
                nc.sync.dma_start(out=wf_sb[:, k * HIDDEN:(k + 1) * HIDDEN],
                                  in_=wf[k * 128:(k + 1) * 128, :])
                nc.sync.dma_start(out=wh_sb[:, k * HIDDEN:(k + 1) * HIDDEN],
                                  in_=wh[k * 128:(k + 1) * 128, :])

            # ---------- phase 1: U table  U_c = relu(emb @ (2 WiT_c) + 2 bi_c) ----------
            for i in range(NVT):
                et = uph.tile([128, EMBED], F16, tag="et")
                nc.sync.dma_start(out=et[:], in_=embt[i * 128:(i + 1) * 128, :])
                pu = upsum.tile([128, HC], F32, tag="pu")
                for e in range(NEC):
                    nc.tensor.matmul(out=pu[:], lhsT=et[:, e * 128:(e + 1) * 128],
                                     rhs=wi_sb[:, e * HC:(e + 1) * HC],
                                     start=(e == 0), stop=False)
                nc.tensor.matmul(out=pu[:], lhsT=ones128[:], rhs=bi_sb[:],
                                 start=False, stop=True)
                u_sb = uph.tile([128, HC], F16, tag="usb")
                nc.scalar.activation(u_sb[:], pu[:], mybir.ActivationFunctionType.Relu)
                nc.scalar.dma_start(out=u_dram[i * 128:(i + 1) * 128, :], in_=u_sb[:])

            # ---------- phase 2: gather inp_c rows (t-major) + phase 3: AllGather ----------
            ng_per_ch = (B * TCH) // 128  # 32 gather calls per time chunk
            ncalls = ntch * ng_per_ch
            idx_all = cst.tile([128, 256], I32, tag="idx_all")
            nc.sync.dma_start(out=idx_all[:, :ncalls], in_=idx[:, 0:ncalls])
            for j in range(ntch):
                for g in range(ng_per_ch):
                    k = j * ng_per_ch + g
                    gt = uph.tile([128, HC], F16, tag="gt")
                    nc.gpsimd.indirect_dma_start(
                        out=gt[:], out_offset=None,
                        in_=u_dram[:, :],
                        in_offset=bass.IndirectOffsetOnAxis(ap=idx_all[:, k:k + 1], axis=0))
                    nc.sync.dma_start(out=agin[j][g * 128:(g + 1) * 128, :], in_=gt[:])
                nc.gpsimd.collective_compute(
                    "AllGather", mybir.AluOpType.bypass,
                    replica_groups=[list(range(NCORES))],
                    ins=[agin[j].opt()], outs=[gath[j].opt()])

            # ---------- phase 4: recurrence in h' = 2h - 1 space ----------
            # psum halves: rows 0:64 = f-gate (-> S' = tanh), rows 64:128 = h-gate (-> T = tanh)
            # h'_new = S' + T * inp2
            hT = []
            for k in range(NKC):
                t0 = rec.tile([128, 64], F16, tag=f"hT{k}")
                nc.vector.memset(t0[:], -1.0)
                hT.append(t0)

            def open_bias(pgA, pgB):
                for half, pgX in ((0, pgA), (1, pgB)):
                    ns = slice(half * 512, (half + 1) * 512)
                    nc.tensor.matmul(out=pgX[0:64, :], lhsT=ones64[:], rhs=bf_sb[:, ns],
                                     start=True, stop=False, tile_position=(0, 0))
                    nc.tensor.matmul(out=pgX[64:128, :], lhsT=ones64[:], rhs=bh_sb[:, ns],
                                     start=True, stop=False, tile_position=(0, 64))

            pgA = gpsum.tile([128, 512], F32, tag="pgA")
            pgB = gpsum.tile([128, 512], F32, tag="pgB")
            open_bias(pgA, pgB)

            for t in range(steps):
                j, tl = t // TCH, t % TCH
                inp2 = inpool.tile([B, HIDDEN], F16, tag="inp")
                src = bass.AP(tensor=gath[j].tensor, offset=tl * B * HC,
                              ap=[[HC, B], [B * TCH * HC, NCORES], [1, HC]])
                nc.sync.dma_start(out=inp2[:], in_=src)

                th = [None, None]
                hn = [None] * 4
                hTn = [None] * NKC
                pgs = (pgA, pgB)
                for half in (0, 1):
                    pg = pgs[half]
                    for k in range(NKC):
                        woff = k * HIDDEN + half * 512
                        nc.tensor.matmul(out=pg[0:64, :], lhsT=hT[k][:, :],
                                         rhs=wf_sb[:, woff:woff + 512],
                                         start=False, stop=(k == NKC - 1),
                                         tile_position=(0, 0))
                        nc.tensor.matmul(out=pg[64:128, :], lhsT=hT[k][:, :],
                                         rhs=wh_sb[:, woff:woff + 512],
                                         start=False, stop=(k == NKC - 1),
                                         tile_position=(0, 64))
                    # activation(s) for this half
                    if DVE_SHIFT:
                        th[half] = rec.tile([128, 512], F16, tag=f"th{half}", name=f"th{half}")
                        nc.scalar.activation(th[half][:, :], pg[:, :], Tanh)
                    else:
                        th[half] = rec.tile([64, 1024], F16, tag=f"th{half}", name=f"th{half}")
                        # T at cols 0:512, S' at cols 512:1024, both partition base 0
                        nc.scalar.activation(th[half][:, 0:512], pg[64:128, :], Tanh)
                        nc.scalar.activation(th[half][:, 512:1024], pg[0:64, :], Tanh)
                    # combine per quarter + transpose chunks via DMA XBAR
                    for qi in (0, 1):
                        q = half * 2 + qi
                        qs = slice(qi * 256, (qi + 1) * 256)
                        hn[q] = rec.tile([64, 256], F16, tag=f"hn{q}", name=f"hn{q}")
                        if DVE_SHIFT:
                            nc.vector.tensor_mul(out=hn[q][:, :], in0=th[half][64:128, qs],
                                                 in1=inp2[:, q * 256:(q + 1) * 256])
                            nc.vector.tensor_add(out=hn[q][:, :], in0=hn[q][:, :],
                                                 in1=th[half][0:64, qs])
                        else:
                            nc.vector.tensor_mul(out=hn[q][:, :], in0=th[half][:, qs],
                                                 in1=inp2[:, q * 256:(q + 1) * 256])
                            nc.vector.tensor_add(out=hn[q][:, :], in0=hn[q][:, :],
                                                 in1=th[half][:, 512 + qs.start:512 + qs.stop])
                        for ki in (0, 1):
                            kk = q * 2 + ki
                            hTn[kk] = rec.tile([128, 64], F16, tag=f"hT{kk}", name=f"hTn{kk}")
                            nc.sync.dma_start(out=hTn[kk][:, :],
                                              in_=hn[q][:, ki * 128:(ki + 1) * 128],
                                              transpose=True)
                        nc.scalar.dma_start(out=ring[t * B:(t + 1) * B, q * 256:(q + 1) * 256],
                                            in_=hn[q][:, :])
                # open next step's psum with the bias rows (fills PE bubble, keeps HAM warm)
                if t < steps - 1:
                    pgA = gpsum.tile([128, 512], F32, tag="pgA")
                    pgB = gpsum.tile([128, 512], F32, tag="pgB")
                    open_bias(pgA, pgB)
                hT = hTn

            # ---------- phase 5: select + linear + log_softmax ----------
            six = cst.tile([128, 1], I32, tag="six")
            nc.sync.dma_start(out=six[:], in_=selidx[:])
            hsel = cst.tile([128, HIDDEN], F16, tag="hsel")
            nc.gpsimd.indirect_dma_start(
                out=hsel[:], out_offset=None,
                in_=ring[:, :],
                in_offset=bass.IndirectOffsetOnAxis(ap=six[:, :1], axis=0))
            # transpose hsel[0:64] -> hselT chunks
            pt2 = tpsum.tile([128, NKC * 64], F16, tag="pt")
            for k in range(NKC):
                nc.tensor.transpose(out=pt2[:, k * 64:(k + 1) * 64],
                                    in_=hsel[0:64, k * 128:(k + 1) * 128],
                                    identity=ident[:])
            hselT = cst.tile([128, NKC * 64], F16, tag="hselT")
            nc.vector.tensor_copy(out=hselT[:], in_=pt2[:])
            # lin = hsel' @ Wo_eff.T + bo_eff
            wo_sb = wpool.tile([128, NKC * HIDDEN], F16, tag="wo")
            for k in range(NKC):
                nc.sync.dma_start(out=wo_sb[:, k * HIDDEN:(k + 1) * HIDDEN],
                                  in_=wo[k * 128:(k + 1) * 128, :])
            plA = gpsum.tile([64, 512], F32, tag="pgA")
            plB = gpsum.tile([64, 512], F32, tag="pgB")
            pls = (plA, plB)
            for k in range(NKC):
                for n in range(2):
                    nc.tensor.matmul(out=pls[n][:, :], lhsT=hselT[:, k * 64:(k + 1) * 64],
                                     rhs=wo_sb[:, k * HIDDEN + n * 512:k * HIDDEN + (n + 1) * 512],
                                     start=(k == 0), stop=False)
            for n in range(2):
                ns = slice(n * 512, (n + 1) * 512)
                nc.tensor.matmul(out=pls[n][:, :], lhsT=ones64[:], rhs=bo_sb[:, ns],
                                 start=False, stop=True)
            lin = cst.tile([64, HIDDEN], F16, tag="lin")
            nc.vector.tensor_copy(out=lin[:, 0:512], in_=plA[:])
            nc.vector.tensor_copy(out=lin[:, 512:1024], in_=plB[:])
            pt3 = tpsum.tile([128, NKC * 64], F16, tag="pt")
            for k in range(NKC):
                nc.tensor.transpose(out=pt3[:, k * 64:(k + 1) * 64],
                                    in_=lin[:, k * 128:(k + 1) * 128],
                                    identity=ident[:])
            linT = cst.tile([128, NKC * 64], F16, tag="linT")
            nc.vector.tensor_copy(out=linT[:], in_=pt3[:])
            wl_sb = cst.tile([128, NKC * NCLS], F16, tag="wl")
            for k in range(NKC):
                nc.sync.dma_start(out=wl_sb[:, k * NCLS:(k + 1) * NCLS],
                                  in_=wlin[k * 128:(k + 1) * 128, :])
            pz = upsum.tile([64, NCLS], F32, tag="pu")
            for k in range(NKC):
                nc.tensor.matmul(out=pz[:], lhsT=linT[:, k * 64:(k + 1) * 64],
                                 rhs=wl_sb[:, k * NCLS:(k + 1) * NCLS],
                                 start=(k == 0), stop=(k == NKC - 1))
            # log_softmax over the 2 classes (free axis)
            m = cst.tile([64, 1], F32, tag="m")
            nc.vector.tensor_reduce(out=m[:], in_=pz[:], axis=mybir.AxisListType.X,
                                    op=mybir.AluOpType.max)
            xm = cst.tile([64, NCLS], F32, tag="xm")
            nc.vector.tensor_scalar(out=xm[:], in0=pz[:], scalar1=m[:], scalar2=None,
                                    op0=mybir.AluOpType.subtract)
            esum = cst.tile([64, 1], F32, tag="esum")
            ex = cst.tile([64, NCLS], F32, tag="ex")
            nc.scalar.activation(ex[:], xm[:], mybir.ActivationFunctionType.Exp,
                                 accum_out=esum[:])
            lns = cst.tile([64, 1], F32, tag="lns")
            nc.scalar.activation(lns[:], esum[:], mybir.ActivationFunctionType.Ln)
            res = cst.tile([64, NCLS], F32, tag="res")
            nc.vector.tensor_scalar(out=res[:], in0=xm[:], scalar1=lns[:], scalar2=None,
                                    op0=mybir.AluOpType.subtract)
            nc.sync.dma_start(out=out_ext[:, :], in_=res[:])

    nc.compile()
    return nc


def _prep(x, lengths, emb, W_i, b_i, W_f, b_f, W_h, b_h, W_o, b_o, W_lin, b_lin,
          steps=S):
    f16 = np.float16
    f32 = np.float32
    # folded weights for the h' = 2h - 1 reformulation (see _build docstring):
    #   psum_f = h' @ (0.25 Wf).T + (0.5 bf + 0.25 rowsum Wf)   -> S' = tanh(psum_f)
    #   psum_h = h' @ (0.50 Wh).T + (bh + 0.5 rowsum Wh)        -> T  = tanh(psum_h)
    #   inp2   = relu(e @ (2 Wi).T + 2 bi)
    #   h'_new = S' + T * inp2
    #   lin    = h'_sel @ (0.5 Wo).T + (bo + 0.5 rowsum Wo)
    W_f = W_f.astype(f32); W_h = W_h.astype(f32); W_o = W_o.astype(f32)
    Wf_eff = 0.25 * W_f
    bf_eff = 0.5 * b_f.astype(f32) + 0.25 * W_f.sum(axis=1)
    Wh_eff = 0.5 * W_h
    bh_eff = b_h.astype(f32) + 0.5 * W_h.sum(axis=1)
    Wi_eff = 2.0 * W_i.astype(f32)
    bi_eff = 2.0 * b_i.astype(f32)
    Wo_eff = 0.5 * W_o
    bo_eff = b_o.astype(f32) + 0.5 * W_o.sum(axis=1)

    # contiguous per-vocab-tile layout: embt2[i*128+p, e*128+c] = emb[i*128+c, e*128+p]
    E = emb.astype(f16)
    embt2 = np.ascontiguousarray(
        E.reshape(NVT, 128, NEC, 128).transpose(0, 3, 2, 1).reshape(NVT * 128, EMBED))
    x_tm = np.ascontiguousarray(x.T)  # [S, B] t-major
    idx_tm = np.ascontiguousarray(x_tm.reshape(TOK // 128, 128).T).astype(np.int32)  # [128, 256] col-major
    sel = ((lengths.astype(np.int64) - 1) * B + np.arange(B)).astype(np.int32)
    selpad = np.zeros((128, 1), np.int32)
    selpad[:B, 0] = sel
    maps = []
    for c in range(NCORES):
        hsl = slice(c * HC, (c + 1) * HC)
        maps.append({
            "embt": embt2,
            "wi": np.ascontiguousarray(Wi_eff[hsl, :].T.astype(f16)),
            "bi": bi_eff[None, hsl].astype(f16),
            "wf": np.ascontiguousarray(Wf_eff.T.astype(f16)),
            "wh": np.ascontiguousarray(Wh_eff.T.astype(f16)),
            "bf_r": bf_eff[None, :].astype(f16),
            "bh_r": bh_eff[None, :].astype(f16),
            "wo": np.ascontiguousarray(Wo_eff.T.astype(f16)),
            "bo_r": bo_eff[None, :].astype(f16),
            "wlin": np.ascontiguousarray(W_lin.T.astype(f16)),
            "idx": idx_tm,
            "selidx": selpad,
        })
    return maps


def _run(inputs, steps=S, trace=False):
    key = steps
    if key not in _CACHE:
        _CACHE[key] = _build(steps)
    nc = _CACHE[key]
    maps = _prep(**inputs, steps=steps)
    res = run_bass_kernel_spmd(nc, maps, core_ids=list(range(NCORES)), trace=trace)
    return res


def kernel(**inputs) -> np.ndarray:
    res = _run(inputs, steps=S, trace=False)
    return res.results[0]["out"]


if __name__ == "__main__":
    steps = int(os.environ.get("KSTEPS", "8"))
    rng = np.random.default_rng(0)
    x = rng.integers(0, VOCAB, size=(B, S)).astype(np.int64)
    lengths = rng.integers(1, steps + 1, size=(B,)).astype(np.int64)
    lengths[0] = steps
    s_e, s_h = 1 / np.sqrt(EMBED), 1 / np.sqrt(HIDDEN)
    ins = dict(
        x=x, lengths=lengths,
        emb=rng.normal(size=(VOCAB, EMBED)).astype(np.float32),
        W_i=rng.uniform(-s_e, s_e, (HIDDEN, EMBED)).astype(np.float32),
        b_i=rng.uniform(-s_e, s_e, (HIDDEN,)).astype(np.float32),
        W_f=rng.uniform(-s_h, s_h, (HIDDEN, HIDDEN)).astype(np.float32),
        b_f=rng.uniform(-s_h, s_h, (HIDDEN,)).astype(np.float32),
        W_h=rng.uniform(-s_h, s_h, (HIDDEN, HIDDEN)).astype(np.float32),
        b_h=rng.uniform(-s_h, s_h, (HIDDEN,)).astype(np.float32),
        W_o=rng.uniform(-s_h, s_h, (HIDDEN, HIDDEN)).astype(np.float32),
        b_o=rng.uniform(-s_h, s_h, (HIDDEN,)).astype(np.float32),
        W_lin=rng.uniform(-s_h, s_h, (NCLS, HIDDEN)).astype(np.float32),
        b_lin=np.zeros((NCLS,), np.float32),
    )
    # numpy reference (on truncated steps)
    def npref(steps):
        e = ins["emb"][x]  # [B, S, E]
        h = np.zeros((B, HIDDEN), np.float32)
        outs = np.zeros((steps, B, HIDDEN), np.float32)
        for t in range(steps):
            et_ = e[:, t, :]
            inp = np.maximum(et_ @ ins["W_i"].T + ins["b_i"], 0)
            hf = 1 / (1 + np.exp(-(h @ ins["W_f"].T + ins["b_f"])))
            hh = np.tanh(h @ ins["W_h"].T + ins["b_h"])
            h = hf + hh * inp
            outs[t] = h
        li = outs[lengths - 1, np.arange(B)]
        lin = li @ ins["W_o"].T + ins["b_o"]
        lg = lin @ ins["W_lin"].T + ins["b_lin"]
        lg = lg - lg.max(1, keepdims=True)
        return lg - np.log(np.exp(lg).sum(1, keepdims=True))

    expected = npref(steps)
    res = _run(ins, steps=steps, trace=False)
    got = res.results[0]["out"]
    err = np.linalg.norm(got - expected) / np.linalg.norm(expected)
    print("expected[:3]:", expected[:3])
    print("got[:3]:", got[:3])
    print("rel_err:", err)


# revision 8
# speedup vs baseline: 1.1404x; 1.1404x over previous
"""Trainium2 Bass kernel for nn_FCLSTM: embedding -> custom LSTM-ish recurrence -> select -> linear -> log_softmax.

Self-contained: hardcodes shapes. kernel(**inputs) takes full numpy inputs, returns [64, 2] fp32.

v2 redesign vs baseline:
  - State transform h' = 2h - 1 turns sigmoid(f)+tanh(g)*inp into tanh(f')+tanh(g')*inp2
    with all scales/biases folded into the weights host-side -> ONE tanh ACT op per
    psum half ([128,512]) instead of sigmoid+tanh pairs.
  - Bias matmuls issued as next-step PSUM group openers (fill the PE pipeline bubble).
  - h_new -> hT transposes moved off the PE onto the DMA XBAR transpose engine.
  - Per-chunk hT tiles so next-step matmuls wait only on their own chunk.
  - Contiguous embedding-table layout (one [128,512] DMA per vocab tile).
"""
import os
import numpy as np

import concourse.bacc as bacc
import concourse.bass as bass
import concourse.mybir as mybir
from concourse import library_config  # noqa: F401
from concourse.tile import TileContext
from concourse.masks import make_identity
from concourse.bass_utils import run_bass_kernel_spmd

VOCAB, EMBED, HIDDEN, NCLS = 32000, 512, 1024, 2
B, S = 64, 512
NCORES = 8
HC = HIDDEN // NCORES          # 128 per-core H slice for the U table
NVT = VOCAB // 128             # 250 vocab tiles
NEC = EMBED // 128             # 4 embed (contraction) chunks
NKC = HIDDEN // 128            # 8 hidden contraction chunks
TCH = S // 8                   # 64 steps per AllGather time-chunk
TOK = B * S                    # 32768 tokens
F16 = mybir.dt.float16
F32 = mybir.dt.float32
I32 = mybir.dt.int32
Tanh = mybir.ActivationFunctionType.Tanh

# Plan A: one tanh per [128,512] psum half; DVE mul reads T at partition base 64.
# Plan B fallback (DVE_SHIFT=False): two tanh ACT ops per half, both landing at base 0
# (partition-shifted ACT reads, baseline-proven).
DVE_SHIFT = False

_CACHE = {}


def _build(steps=S):
    nc = bacc.Bacc("TRN2", target_bir_lowering=False, debug=False, num_devices=NCORES)

    # ---------- inputs ----------
    embt = nc.dram_tensor("embt", [NVT * 128, EMBED], F16, kind="ExternalInput")
    wi = nc.dram_tensor("wi", [EMBED, HC], F16, kind="ExternalInput")
    bi = nc.dram_tensor("bi", [1, HC], F16, kind="ExternalInput")
    wf = nc.dram_tensor("wf", [HIDDEN, HIDDEN], F16, kind="ExternalInput")
    wh = nc.dram_tensor("wh", [HIDDEN, HIDDEN], F16, kind="ExternalInput")
    bf_r = nc.dram_tensor("bf_r", [1, HIDDEN], F16, kind="ExternalInput")
    bh_r = nc.dram_tensor("bh_r", [1, HIDDEN], F16, kind="ExternalInput")
    wo = nc.dram_tensor("wo", [HIDDEN, HIDDEN], F16, kind="ExternalInput")
    bo_r = nc.dram_tensor("bo_r", [1, HIDDEN], F16, kind="ExternalInput")
    wlin = nc.dram_tensor("wlin", [HIDDEN, NCLS], F16, kind="ExternalInput")
    idx = nc.dram_tensor("idx", [128, TOK // 128], I32, kind="ExternalInput")
    selidx = nc.dram_tensor("selidx", [128, 1], I32, kind="ExternalInput")
    out_ext = nc.dram_tensor("out", [B, NCLS], F32, kind="ExternalOutput")

    ntch = (steps + TCH - 1) // TCH  # number of time chunks actually used

    with TileContext(nc) as tc:
        with (
            tc.tile_pool(name="dram", bufs=1, space="DRAM") as dram,
            tc.tile_pool(name="const", bufs=1) as cst,
            tc.tile_pool(name="w", bufs=1) as wpool,
            tc.tile_pool(name="uph", bufs=4) as uph,
            tc.tile_pool(name="upsum", bufs=2, space="PSUM") as upsum,
            tc.tile_pool(name="rec", bufs=2) as rec,
            tc.tile_pool(name="inp", bufs=3) as inpool,
            tc.tile_pool(name="gpsum", bufs=2, space="PSUM") as gpsum,
            tc.tile_pool(name="tpsum", bufs=1, space="PSUM") as tpsum,
        ):
            # ---------- DRAM scratch ----------
            u_dram = dram.tile([VOCAB, HC], F16)
            agin = [dram.tile([B * TCH, HC], F16, name=f"agin{j}") for j in range(ntch)]
            gath = [dram.tile([NCORES * B * TCH, HC], F16, name=f"gath{j}", addr_space="Shared") for j in range(ntch)]
            ring = dram.tile([TOK, HIDDEN], F16)

            # ---------- constants / weights to SBUF ----------
            ones64 = cst.tile([1, 64], F16, tag="ones64")
            nc.vector.memset(ones64[:], 1.0)
            ones128 = cst.tile([1, 128], F16, tag="ones128")
            nc.vector.memset(ones128[:], 1.0)
            ident = cst.tile([64, 64], F16, tag="ident")
            make_identity(nc, ident[:])

            wi_sb = cst.tile([128, NEC * HC], F16, tag="wi")
            for e in range(NEC):
                nc.sync.dma_start(out=wi_sb[:, e * HC:(e + 1) * HC],
                                  in_=wi[e * 128:(e + 1) * 128, :])
            bi_sb = cst.tile([1, HC], F16, tag="bi")
            nc.sync.dma_start(out=bi_sb[:], in_=bi[:])
            bf_sb = cst.tile([1, HIDDEN], F16, tag="bf")
            nc.sync.dma_start(out=bf_sb[:], in_=bf_r[:])
            bh_sb = cst.tile([1, HIDDEN], F16, tag="bh")
            nc.sync.dma_start(out=bh_sb[:], in_=bh_r[:])
            bo_sb = cst.tile([1, HIDDEN], F16, tag="bo")
            nc.sync.dma_start(out=bo_sb[:], in_=bo_r[:])

            wf_sb = wpool.tile([128, NKC * HIDDEN], F16, tag="wf")
            wh_sb = wpool.tile([128, NKC * HIDDEN], F16, tag="wh")
            for k in range(NKC):
                nc.sync.dma_start(out=wf_sb[:, k * HIDDEN:(k + 1) * HIDDEN],
                                  in_=wf[k * 128:(k + 1) * 128, :])
                nc.sync.dma_start(out=wh_sb[:, k * HIDDEN:(k + 1) * HIDDEN],
                                  in_=wh[k * 128:(k + 1) * 128, :])

            # ---------- phase 1: U table  U_c = relu(emb @ (2 WiT_c) + 2 bi_c) ----------
            for i in range(NVT):
                et = uph.tile([128, EMBED], F16, tag="et")
                nc.sync.dma_start(out=et[:], in_=embt[i * 128:(i + 1) * 128, :])
                pu = upsum.tile([128, HC], F32, tag="pu")
                for e in range(NEC):
                    nc.tensor.matmul(out=pu[:], lhsT=et[:, e * 128:(e + 1) * 128],
                                     rhs=wi_sb[:, e * HC:(e + 1) * HC],
                                     start=(e == 0), stop=False)
                nc.tensor.matmul(out=pu[:], lhsT=ones128[:], rhs=bi_sb[:],
                                 start=False, stop=True)
                u_sb = uph.tile([128, HC], F16, tag="usb")
                nc.scalar.activation(u_sb[:], pu[:], mybir.ActivationFunctionType.Relu)
                nc.scalar.dma_start(out=u_dram[i * 128:(i + 1) * 128, :], in_=u_sb[:])

            # ---------- phase 2: gather inp_c rows (t-major) + phase 3: AllGather ----------
            ng_per_ch = (B * TCH) // 128  # 32 gather calls per time chunk
            ncalls = ntch * ng_per_ch
            idx_all = cst.tile([128, 256], I32, tag="idx_all")
            nc.sync.dma_start(out=idx_all[:, :ncalls], in_=idx[:, 0:ncalls])
            for j in range(ntch):
                for g in range(ng_per_ch):
                    k = j * ng_per_ch + g
                    gt = uph.tile([128, HC], F16, tag="gt")
                    nc.gpsimd.indirect_dma_start(
                        out=gt[:], out_offset=None,
                        in_=u_dram[:, :],
                        in_offset=bass.IndirectOffsetOnAxis(ap=idx_all[:, k:k + 1], axis=0))
                    nc.sync.dma_start(out=agin[j][g * 128:(g + 1) * 128, :], in_=gt[:])
                nc.gpsimd.collective_compute(
                    "AllGather", mybir.AluOpType.bypass,
                    replica_groups=[list(range(NCORES))],
                    ins=[agin[j].opt()], outs=[gath[j].opt()])

            # ---------- phase 4: recurrence in h' = 2h - 1 space ----------
            # psum halves: rows 0:64 = f-gate (-> S' = tanh), rows 64:128 = h-gate (-> T = tanh)
            # h'_new = S' + T * inp2
            # Steps are processed in PAIRS sharing one inp2 tile: even step t=2m computes
            # its tail on partitions 0:64, odd step on partitions 64:128 (ACT does the
            # partition shift; DVE ops stay base-aligned). inp2 pair tiles are loaded with
            # 8 contiguous [128,128] DMAs (one per core-slice of the gathered U table).
            hT = []
            for k in range(NKC):
                t0 = rec.tile([128, 64], F16, tag=f"hT{k}")
                nc.vector.memset(t0[:], -1.0)
                hT.append(t0)

            def open_bias(pgA, pgB):
                for half, pgX in ((0, pgA), (1, pgB)):
                    ns = slice(half * 512, (half + 1) * 512)
                    nc.tensor.matmul(out=pgX[0:64, :], lhsT=ones64[:], rhs=bf_sb[:, ns],
                                     start=True, stop=False, tile_position=(0, 0))
                    nc.tensor.matmul(out=pgX[64:128, :], lhsT=ones64[:], rhs=bh_sb[:, ns],
                                     start=True, stop=False, tile_position=(0, 64))

            pgA = gpsum.tile([128, 512], F32, tag="pgA")
            pgB = gpsum.tile([128, 512], F32, tag="pgB")
            open_bias(pgA, pgB)

            inp2 = None
            hnp = [None, None]
            for t in range(steps):
                j, tl = t // TCH, t % TCH
                par = t % 2          # 0: rows 0:64, 1: rows 64:128
                rs = slice(64 * par, 64 * par + 64)
                if par == 0:
                    # load inp2 for this step pair: 8 contiguous [128,128] reads
                    nrow = 128 if t + 1 < steps else 64
                    inp2 = inpool.tile([128, HIDDEN], F16, tag="inp", name="inp2")
                    for c in range(NCORES):
                        src = bass.AP(tensor=gath[j].tensor,
                                      offset=c * B * TCH * HC + tl * B * HC,
                                      ap=[[HC, nrow], [1, HC]])
                        nc.sync.dma_start(out=inp2[0:nrow, c * HC:(c + 1) * HC], in_=src)
                    hnp = [rec.tile([128, 512], F16, tag=f"hnp{h}", name=f"hnp{h}") for h in (0, 1)]

                th = [None, None]
                hTn = [None] * NKC
                pgs = (pgA, pgB)
                for half in (0, 1):
                    pg = pgs[half]
                    for k in range(NKC):
                        woff = k * HIDDEN + half * 512
                        nc.tensor.matmul(out=pg[0:64, :], lhsT=hT[k][:, :],
                                         rhs=wf_sb[:, woff:woff + 512],
                                         start=False, stop=(k == NKC - 1),
                                         tile_position=(0, 0))
                        nc.tensor.matmul(out=pg[64:128, :], lhsT=hT[k][:, :],
                                         rhs=wh_sb[:, woff:woff + 512],
                                         start=False, stop=(k == NKC - 1),
                                         tile_position=(0, 64))
                    # T at cols 0:512, S' at cols 512:1024, on this parity's partition rows
                    th[half] = rec.tile([128, 1024], F16, tag=f"th{half}", name=f"th{half}")
                    nc.scalar.activation(th[half][rs, 0:512], pg[64:128, :], Tanh)
                    nc.scalar.activation(th[half][rs, 512:1024], pg[0:64, :], Tanh)
                    # combine on this parity's rows (all operands base-aligned)
                    hs = slice(half * 512, (half + 1) * 512)
                    nc.vector.tensor_mul(out=hnp[half][rs, :], in0=th[half][rs, 0:512],
                                         in1=inp2[rs, hs])
                    nc.vector.tensor_add(out=hnp[half][rs, :], in0=hnp[half][rs, :],
                                         in1=th[half][rs, 512:1024])
                    # hT chunks for the next step via DMA XBAR transpose
                    for ki in range(4):
                        kk = half * 4 + ki
                        hTn[kk] = rec.tile([128, 64], F16, tag=f"hT{kk}", name=f"hTn{kk}")
                        nc.sync.dma_start(out=hTn[kk][:, :],
                                          in_=hnp[half][rs, ki * 128:(ki + 1) * 128],
                                          transpose=True)
                if par == 1 or t == steps - 1:
                    # one ring write per step pair per half ([t*B-64 .. t*B+64) rows)
                    nr = 128 if par == 1 else 64
                    r0 = (t - par) * B
                    for half in (0, 1):
                        nc.scalar.dma_start(
                            out=ring[r0:r0 + nr, half * 512:(half + 1) * 512],
                            in_=hnp[half][0:nr, :])
                # open next step's psum with the bias rows (fills PE bubble, keeps HAM warm)
                if t < steps - 1:
                    pgA = gpsum.tile([128, 512], F32, tag="pgA")
                    pgB = gpsum.tile([128, 512], F32, tag="pgB")
                    open_bias(pgA, pgB)
                hT = hTn

            # ---------- phase 5: select + linear + log_softmax ----------
            six = cst.tile([128, 1], I32, tag="six")
            nc.sync.dma_start(out=six[:], in_=selidx[:])
            hsel = cst.tile([128, HIDDEN], F16, tag="hsel")
            nc.gpsimd.indirect_dma_start(
                out=hsel[:], out_offset=None,
                in_=ring[:, :],
                in_offset=bass.IndirectOffsetOnAxis(ap=six[:, :1], axis=0))
            # transpose hsel[0:64] -> hselT chunks
            pt2 = tpsum.tile([128, NKC * 64], F16, tag="pt")
            for k in range(NKC):
                nc.tensor.transpose(out=pt2[:, k * 64:(k + 1) * 64],
                                    in_=hsel[0:64, k * 128:(k + 1) * 128],
                                    identity=ident[:])
            hselT = cst.tile([128, NKC * 64], F16, tag="hselT")
            nc.vector.tensor_copy(out=hselT[:], in_=pt2[:])
            # lin = hsel' @ Wo_eff.T + bo_eff
            wo_sb = wpool.tile([128, NKC * HIDDEN], F16, tag="wo")
            for k in range(NKC):
                nc.sync.dma_start(out=wo_sb[:, k * HIDDEN:(k + 1) * HIDDEN],
                                  in_=wo[k * 128:(k + 1) * 128, :])
            plA = gpsum.tile([64, 512], F32, tag="pgA")
            plB = gpsum.tile([64, 512], F32, tag="pgB")
            pls = (plA, plB)
            for k in range(NKC):
                for n in range(2):
                    nc.tensor.matmul(out=pls[n][:, :], lhsT=hselT[:, k * 64:(k + 1) * 64],
                                     rhs=wo_sb[:, k * HIDDEN + n * 512:k * HIDDEN + (n + 1) * 512],
                                     start=(k == 0), stop=False)
            for n in range(2):
                ns = slice(n * 512, (n + 1) * 512)
                nc.tensor.matmul(out=pls[n][:, :], lhsT=ones64[:], rhs=bo_sb[:, ns],
                                 start=False, stop=True)
            lin = cst.tile([64, HIDDEN], F16, tag="lin")
            nc.vector.tensor_copy(out=lin[:, 0:512], in_=plA[:])
            nc.vector.tensor_copy(out=lin[:, 512:1024], in_=plB[:])
            pt3 = tpsum.tile([128, NKC * 64], F16, tag="pt")
            for k in range(NKC):
                nc.tensor.transpose(out=pt3[:, k * 64:(k + 1) * 64],
                                    in_=lin[:, k * 128:(k + 1) * 128],
                                    identity=ident[:])
            linT = cst.tile([128, NKC * 64], F16, tag="linT")
            nc.vector.tensor_copy(out=linT[:], in_=pt3[:])
            wl_sb = cst.tile([128, NKC * NCLS], F16, tag="wl")
            for k in range(NKC):
                nc.sync.dma_start(out=wl_sb[:, k * NCLS:(k + 1) * NCLS],
                                  in_=wlin[k * 128:(k + 1) * 128, :])
            pz = upsum.tile([64, NCLS], F32, tag="pu")
            for k in range(NKC):
                nc.tensor.matmul(out=pz[:], lhsT=linT[:, k * 64:(k + 1) * 64],
                                 rhs=wl_sb[:, k * NCLS:(k + 1) * NCLS],
                                 start=(k == 0), stop=(k == NKC - 1))
            # log_softmax over the 2 classes (free axis)
            m = cst.tile([64, 1], F32, tag="m")
            nc.vector.tensor_reduce(out=m[:], in_=pz[:], axis=mybir.AxisListType.X,
                                    op=mybir.AluOpType.max)
            xm = cst.tile([64, NCLS], F32, tag="xm")
            nc.vector.tensor_scalar(out=xm[:], in0=pz[:], scalar1=m[:], scalar2=None,
                                    op0=mybir.AluOpType.subtract)
            esum = cst.tile([64, 1], F32, tag="esum")
            ex = cst.tile([64, NCLS], F32, tag="ex")
            nc.scalar.activation(ex[:], xm[:], mybir.ActivationFunctionType.Exp,
                                 accum_out=esum[:])
            lns = cst.tile([64, 1], F32, tag="lns")
            nc.scalar.activation(lns[:], esum[:], mybir.ActivationFunctionType.Ln)
            res = cst.tile([64, NCLS], F32, tag="res")
            nc.vector.tensor_scalar(out=res[:], in0=xm[:], scalar1=lns[:], scalar2=None,
                                    op0=mybir.AluOpType.subtract)
            nc.sync.dma_start(out=out_ext[:, :], in_=res[:])

    nc.compile()
    return nc


def _prep(x, lengths, emb, W_i, b_i, W_f, b_f, W_h, b_h, W_o, b_o, W_lin, b_lin,
          steps=S):
    f16 = np.float16
    f32 = np.float32
    # folded weights for the h' = 2h - 1 reformulation (see _build docstring):
    #   psum_f = h' @ (0.25 Wf).T + (0.5 bf + 0.25 rowsum Wf)   -> S' = tanh(psum_f)
    #   psum_h = h' @ (0.50 Wh).T + (bh + 0.5 rowsum Wh)        -> T  = tanh(psum_h)
    #   inp2   = relu(e @ (2 Wi).T + 2 bi)
    #   h'_new = S' + T * inp2
    #   lin    = h'_sel @ (0.5 Wo).T + (bo + 0.5 rowsum Wo)
    W_f = W_f.astype(f32); W_h = W_h.astype(f32); W_o = W_o.astype(f32)
    Wf_eff = 0.25 * W_f
    bf_eff = 0.5 * b_f.astype(f32) + 0.25 * W_f.sum(axis=1)
    Wh_eff = 0.5 * W_h
    bh_eff = b_h.astype(f32) + 0.5 * W_h.sum(axis=1)
    Wi_eff = 2.0 * W_i.astype(f32)
    bi_eff = 2.0 * b_i.astype(f32)
    Wo_eff = 0.5 * W_o
    bo_eff = b_o.astype(f32) + 0.5 * W_o.sum(axis=1)

    # contiguous per-vocab-tile layout: embt2[i*128+p, e*128+c] = emb[i*128+c, e*128+p]
    E = emb.astype(f16)
    embt2 = np.ascontiguousarray(
        E.reshape(NVT, 128, NEC, 128).transpose(0, 3, 2, 1).reshape(NVT * 128, EMBED))
    x_tm = np.ascontiguousarray(x.T)  # [S, B] t-major
    idx_tm = np.ascontiguousarray(x_tm.reshape(TOK // 128, 128).T).astype(np.int32)  # [128, 256] col-major
    sel = ((lengths.astype(np.int64) - 1) * B + np.arange(B)).astype(np.int32)
    selpad = np.zeros((128, 1), np.int32)
    selpad[:B, 0] = sel
    maps = []
    for c in range(NCORES):
        hsl = slice(c * HC, (c + 1) * HC)
        maps.append({
            "embt": embt2,
            "wi": np.ascontiguousarray(Wi_eff[hsl, :].T.astype(f16)),
            "bi": bi_eff[None, hsl].astype(f16),
            "wf": np.ascontiguousarray(Wf_eff.T.astype(f16)),
            "wh": np.ascontiguousarray(Wh_eff.T.astype(f16)),
            "bf_r": bf_eff[None, :].astype(f16),
            "bh_r": bh_eff[None, :].astype(f16),
            "wo": np.ascontiguousarray(Wo_eff.T.astype(f16)),
            "bo_r": bo_eff[None, :].astype(f16),
            "wlin": np.ascontiguousarray(W_lin.T.astype(f16)),
            "idx": idx_tm,
            "selidx": selpad,
        })
    return maps


def _run(inputs, steps=S, trace=False):
    key = steps
    if key not in _CACHE:
        _CACHE[key] = _build(steps)
    nc = _CACHE[key]
    maps = _prep(**inputs, steps=steps)
    res = run_bass_kernel_spmd(nc, maps, core_ids=list(range(NCORES)), trace=trace)
    return res


def kernel(**inputs) -> np.ndarray:
    res = _run(inputs, steps=S, trace=False)
    return res.results[0]["out"]


if __name__ == "__main__":
    steps = int(os.environ.get("KSTEPS", "8"))
    rng = np.random.default_rng(0)
    x = rng.integers(0, VOCAB, size=(B, S)).astype(np.int64)
    lengths = rng.integers(1, steps + 1, size=(B,)).astype(np.int64)
    lengths[0] = steps
    s_e, s_h = 1 / np.sqrt(EMBED), 1 / np.sqrt(HIDDEN)
    ins = dict(
        x=x, lengths=lengths,
        emb=rng.normal(size=(VOCAB, EMBED)).astype(np.float32),
        W_i=rng.uniform(-s_e, s_e, (HIDDEN, EMBED)).astype(np.float32),
        b_i=rng.uniform(-s_e, s_e, (HIDDEN,)).astype(np.float32),
        W_f=rng.uniform(-s_h, s_h, (HIDDEN, HIDDEN)).astype(np.float32),
        b_f=rng.uniform(-s_h, s_h, (HIDDEN,)).astype(np.float32),
        W_h=rng.uniform(-s_h, s_h, (HIDDEN, HIDDEN)).astype(np.float32),
        b_h=rng.uniform(-s_h, s_h, (HIDDEN,)).astype(np.float32),
        W_o=rng.uniform(-s_h, s_h, (HIDDEN, HIDDEN)).astype(np.float32),
        b_o=rng.uniform(-s_h, s_h, (HIDDEN,)).astype(np.float32),
        W_lin=rng.uniform(-s_h, s_h, (NCLS, HIDDEN)).astype(np.float32),
        b_lin=np.zeros((NCLS,), np.float32),
    )
    # numpy reference (on truncated steps)
    def npref(steps):
        e = ins["emb"][x]  # [B, S, E]
        h = np.zeros((B, HIDDEN), np.float32)
        outs = np.zeros((steps, B, HIDDEN), np.float32)
        for t in range(steps):
            et_ = e[:, t, :]
            inp = np.maximum(et_ @ ins["W_i"].T + ins["b_i"], 0)
            hf = 1 / (1 + np.exp(-(h @ ins["W_f"].T + ins["b_f"])))
            hh = np.tanh(h @ ins["W_h"].T + ins["b_h"])
            h = hf + hh * inp
            outs[t] = h
        li = outs[lengths - 1, np.arange(B)]
        lin = li @ ins["W_o"].T + ins["b_o"]
        lg = lin @ ins["W_lin"].T + ins["b_lin"]
        lg = lg - lg.max(1, keepdims=True)
        return lg - np.log(np.exp(lg).sum(1, keepdims=True))

    expected = npref(steps)
    res = _run(ins, steps=steps, trace=False)
    got = res.results[0]["out"]
    err = np.linalg.norm(got - expected) / np.linalg.norm(expected)
    print("expected[:3]:", expected[:3])
    print("got[:3]:", got[:3])
    print("rel_err:", err)


# revision 9
# speedup vs baseline: 1.6606x; 1.4561x over previous
"""Trainium2 Bass kernel for nn_FCLSTM: embedding -> custom LSTM-ish recurrence -> select -> linear -> log_softmax.

Self-contained: hardcodes shapes. kernel(**inputs) takes full numpy inputs, returns [64, 2] fp32.

v2 redesign vs baseline:
  - State transform h' = 2h - 1 turns sigmoid(f)+tanh(g)*inp into tanh(f')+tanh(g')*inp2
    with all scales/biases folded into the weights host-side -> ONE tanh ACT op per
    psum half ([128,512]) instead of sigmoid+tanh pairs.
  - Bias matmuls issued as next-step PSUM group openers (fill the PE pipeline bubble).
  - h_new -> hT transposes moved off the PE onto the DMA XBAR transpose engine.
  - Per-chunk hT tiles so next-step matmuls wait only on their own chunk.
  - Contiguous embedding-table layout (one [128,512] DMA per vocab tile).
"""
import os
import numpy as np

import concourse.bacc as bacc
import concourse.bass as bass
import concourse.mybir as mybir
from concourse import library_config  # noqa: F401
from concourse.tile import TileContext
from concourse.masks import make_identity
from concourse.bass_utils import run_bass_kernel_spmd

VOCAB, EMBED, HIDDEN, NCLS = 32000, 512, 1024, 2
B, S = 64, 512
NCORES = 8
HC = HIDDEN // NCORES          # 128 per-core H slice for the U table
NVT = VOCAB // 128             # 250 vocab tiles
NEC = EMBED // 128             # 4 embed (contraction) chunks
NKC = HIDDEN // 128            # 8 hidden contraction chunks
TCH = S // 8                   # 64 steps per AllGather time-chunk
TOK = B * S                    # 32768 tokens
F16 = mybir.dt.float16
F32 = mybir.dt.float32
I32 = mybir.dt.int32
Tanh = mybir.ActivationFunctionType.Tanh

# Plan A: one tanh per [128,512] psum half; DVE mul reads T at partition base 64.
# Plan B fallback (DVE_SHIFT=False): two tanh ACT ops per half, both landing at base 0
# (partition-shifted ACT reads, baseline-proven).
DVE_SHIFT = False

_CACHE = {}


def _build(steps=S):
    nc = bacc.Bacc("TRN2", target_bir_lowering=False, debug=False, num_devices=NCORES)

    # ---------- inputs ----------
    embt = nc.dram_tensor("embt", [NVT * 128, EMBED], F16, kind="ExternalInput")
    wi = nc.dram_tensor("wi", [EMBED, HC], F16, kind="ExternalInput")
    bi = nc.dram_tensor("bi", [1, HC], F16, kind="ExternalInput")
    wf = nc.dram_tensor("wf", [HIDDEN, HIDDEN], F16, kind="ExternalInput")
    wh = nc.dram_tensor("wh", [HIDDEN, HIDDEN], F16, kind="ExternalInput")
    bf_r = nc.dram_tensor("bf_r", [1, HIDDEN], F16, kind="ExternalInput")
    bh_r = nc.dram_tensor("bh_r", [1, HIDDEN], F16, kind="ExternalInput")
    wo = nc.dram_tensor("wo", [HIDDEN, HIDDEN], F16, kind="ExternalInput")
    bo_r = nc.dram_tensor("bo_r", [1, HIDDEN], F16, kind="ExternalInput")
    wlin = nc.dram_tensor("wlin", [HIDDEN, NCLS], F16, kind="ExternalInput")
    idx = nc.dram_tensor("idx", [128, TOK // 128], I32, kind="ExternalInput")
    selidx = nc.dram_tensor("selidx", [128, 1], I32, kind="ExternalInput")
    out_ext = nc.dram_tensor("out", [B, NCLS], F32, kind="ExternalOutput")

    ntch = (steps + TCH - 1) // TCH  # number of time chunks actually used

    with TileContext(nc) as tc:
        with (
            tc.tile_pool(name="dram", bufs=1, space="DRAM") as dram,
            tc.tile_pool(name="const", bufs=1) as cst,
            tc.tile_pool(name="w", bufs=1) as wpool,
            tc.tile_pool(name="uph", bufs=4) as uph,
            tc.tile_pool(name="upsum", bufs=2, space="PSUM") as upsum,
            tc.tile_pool(name="rec", bufs=2) as rec,
            tc.tile_pool(name="inp", bufs=3) as inpool,
            tc.tile_pool(name="gpsum", bufs=2, space="PSUM") as gpsum,
            tc.tile_pool(name="tpsum", bufs=1, space="PSUM") as tpsum,
        ):
            # ---------- DRAM scratch ----------
            u_dram = dram.tile([VOCAB, HC], F16)
            agin = [dram.tile([B * TCH, HC], F16, name=f"agin{j}") for j in range(ntch)]
            gath = [dram.tile([NCORES * B * TCH, HC], F16, name=f"gath{j}", addr_space="Shared") for j in range(ntch)]
            ring = dram.tile([TOK, HIDDEN], F16)

            # ---------- constants / weights to SBUF ----------
            ones64 = cst.tile([1, 64], F16, tag="ones64")
            nc.vector.memset(ones64[:], 1.0)
            ones128 = cst.tile([1, 128], F16, tag="ones128")
            nc.vector.memset(ones128[:], 1.0)
            ident = cst.tile([64, 64], F16, tag="ident")
            make_identity(nc, ident[:])

            wi_sb = cst.tile([128, NEC * HC], F16, tag="wi")
            for e in range(NEC):
                nc.sync.dma_start(out=wi_sb[:, e * HC:(e + 1) * HC],
                                  in_=wi[e * 128:(e + 1) * 128, :])
            bi_sb = cst.tile([1, HC], F16, tag="bi")
            nc.sync.dma_start(out=bi_sb[:], in_=bi[:])
            bf_sb = cst.tile([1, HIDDEN], F16, tag="bf")
            nc.sync.dma_start(out=bf_sb[:], in_=bf_r[:])
            bh_sb = cst.tile([1, HIDDEN], F16, tag="bh")
            nc.sync.dma_start(out=bh_sb[:], in_=bh_r[:])
            bo_sb = cst.tile([1, HIDDEN], F16, tag="bo")
            nc.sync.dma_start(out=bo_sb[:], in_=bo_r[:])

            wf_sb = wpool.tile([128, NKC * HIDDEN], F16, tag="wf")
            wh_sb = wpool.tile([128, NKC * HIDDEN], F16, tag="wh")
            for k in range(NKC):
                nc.sync.dma_start(out=wf_sb[:, k * HIDDEN:(k + 1) * HIDDEN],
                                  in_=wf[k * 128:(k + 1) * 128, :])
                nc.sync.dma_start(out=wh_sb[:, k * HIDDEN:(k + 1) * HIDDEN],
                                  in_=wh[k * 128:(k + 1) * 128, :])

            # ---------- phase 1: U table  U_c = relu(emb @ (2 WiT_c) + 2 bi_c) ----------
            for i in range(NVT):
                et = uph.tile([128, EMBED], F16, tag="et")
                nc.sync.dma_start(out=et[:], in_=embt[i * 128:(i + 1) * 128, :])
                pu = upsum.tile([128, HC], F32, tag="pu")
                for e in range(NEC):
                    nc.tensor.matmul(out=pu[:], lhsT=et[:, e * 128:(e + 1) * 128],
                                     rhs=wi_sb[:, e * HC:(e + 1) * HC],
                                     start=(e == 0), stop=False)
                nc.tensor.matmul(out=pu[:], lhsT=ones128[:], rhs=bi_sb[:],
                                 start=False, stop=True)
                u_sb = uph.tile([128, HC], F16, tag="usb")
                nc.scalar.activation(u_sb[:], pu[:], mybir.ActivationFunctionType.Relu)
                nc.scalar.dma_start(out=u_dram[i * 128:(i + 1) * 128, :], in_=u_sb[:])

            # ---------- phase 2: gather inp_c rows (t-major) + phase 3: AllGather ----------
            ng_per_ch = (B * TCH) // 128  # 32 gather calls per time chunk
            ncalls = ntch * ng_per_ch
            idx_all = cst.tile([128, 256], I32, tag="idx_all")
            nc.sync.dma_start(out=idx_all[:, :ncalls], in_=idx[:, 0:ncalls])
            for j in range(ntch):
                for g in range(ng_per_ch):
                    k = j * ng_per_ch + g
                    gt = uph.tile([128, HC], F16, tag="gt")
                    nc.gpsimd.indirect_dma_start(
                        out=gt[:], out_offset=None,
                        in_=u_dram[:, :],
                        in_offset=bass.IndirectOffsetOnAxis(ap=idx_all[:, k:k + 1], axis=0))
                    nc.sync.dma_start(out=agin[j][g * 128:(g + 1) * 128, :], in_=gt[:])
                nc.gpsimd.collective_compute(
                    "AllGather", mybir.AluOpType.bypass,
                    replica_groups=[list(range(NCORES))],
                    ins=[agin[j].opt()], outs=[gath[j].opt()])

            # ---------- phase 4: recurrence in h' = 2h - 1 space ----------
            # psum halves: rows 0:64 = f-gate (-> S' = tanh), rows 64:128 = h-gate (-> T = tanh)
            # h'_new = S' + T * inp2
            # Steps are processed in PAIRS sharing one inp2 tile: even step t=2m computes
            # its tail on partitions 0:64, odd step on partitions 64:128 (ACT does the
            # partition shift; DVE ops stay base-aligned). inp2 pair tiles are loaded with
            # 8 contiguous [128,128] DMAs (one per core-slice of the gathered U table).
            hT = []
            for g in range(2):
                t0 = rec.tile([128, 256], F16, tag=f"hTg{g}")
                nc.vector.memset(t0[:], -1.0)
                hT.append(t0)

            def open_bias(pgA, pgB):
                for half, pgX in ((0, pgA), (1, pgB)):
                    ns = slice(half * 512, (half + 1) * 512)
                    nc.tensor.matmul(out=pgX[0:64, :], lhsT=ones64[:], rhs=bf_sb[:, ns],
                                     start=True, stop=False, tile_position=(0, 0))
                    nc.tensor.matmul(out=pgX[64:128, :], lhsT=ones64[:], rhs=bh_sb[:, ns],
                                     start=True, stop=False, tile_position=(0, 64))

            pgA = gpsum.tile([128, 512], F32, tag="pgA")
            pgB = gpsum.tile([128, 512], F32, tag="pgB")
            open_bias(pgA, pgB)

            inp2 = None
            hnp = [None, None]
            for t in range(steps):
                j, tl = t // TCH, t % TCH
                par = t % 2          # 0: rows 0:64, 1: rows 64:128
                rs = slice(64 * par, 64 * par + 64)
                if par == 0:
                    # load inp2 for this step pair: 8 contiguous [128,128] reads
                    nrow = 128 if t + 1 < steps else 64
                    inp2 = inpool.tile([128, HIDDEN], F16, tag="inp", name="inp2")
                    for c in range(NCORES):
                        src = bass.AP(tensor=gath[j].tensor,
                                      offset=c * B * TCH * HC + tl * B * HC,
                                      ap=[[HC, nrow], [1, HC]])
                        eng = nc.sync if c % 2 == 0 else nc.gpsimd
                        eng.dma_start(out=inp2[0:nrow, c * HC:(c + 1) * HC], in_=src)
                    hnp = [rec.tile([128, 512], F16, tag=f"hnp{h}", name=f"hnp{h}") for h in (0, 1)]

                th = [None, None]
                hTn = [None, None]
                pgs = (pgA, pgB)
                for half in (0, 1):
                    pg = pgs[half]
                    for k in range(NKC):
                        woff = k * HIDDEN + half * 512
                        lhs = hT[k // 4][:, (k % 4) * 64:(k % 4 + 1) * 64]
                        nc.tensor.matmul(out=pg[0:64, :], lhsT=lhs,
                                         rhs=wf_sb[:, woff:woff + 512],
                                         start=False, stop=(k == NKC - 1),
                                         tile_position=(0, 0))
                        nc.tensor.matmul(out=pg[64:128, :], lhsT=lhs,
                                         rhs=wh_sb[:, woff:woff + 512],
                                         start=False, stop=(k == NKC - 1),
                                         tile_position=(0, 64))
                    # T at cols 0:512, S' at cols 512:1024, on this parity's partition rows
                    th[half] = rec.tile([128, 1024], F16, tag=f"th{half}", name=f"th{half}")
                    nc.scalar.activation(th[half][rs, 0:512], pg[64:128, :], Tanh)
                    nc.scalar.activation(th[half][rs, 512:1024], pg[0:64, :], Tanh)
                    # combine on this parity's rows (all operands base-aligned)
                    hs = slice(half * 512, (half + 1) * 512)
                    nc.vector.tensor_mul(out=hnp[half][rs, :], in0=th[half][rs, 0:512],
                                         in1=inp2[rs, hs])
                    nc.vector.tensor_add(out=hnp[half][rs, :], in0=hnp[half][rs, :],
                                         in1=th[half][rs, 512:1024])
                    # hT chunk-group for the next step via one wide DMA XBAR transpose
                    hTn[half] = rec.tile([128, 256], F16, tag=f"hTg{half}", name=f"hTn{half}")
                    nc.sync.dma_start(out=hTn[half][:, :].rearrange("p (c i) -> p c i", c=4),
                                      in_=hnp[half][rs, :],
                                      transpose=True)
                if par == 1 or t == steps - 1:
                    # one ring write per step pair per half ([t*B-64 .. t*B+64) rows)
                    nr = 128 if par == 1 else 64
                    r0 = (t - par) * B
                    for half in (0, 1):
                        nc.scalar.dma_start(
                            out=ring[r0:r0 + nr, half * 512:(half + 1) * 512],
                            in_=hnp[half][0:nr, :])
                # open next step's psum with the bias rows (fills PE bubble, keeps HAM warm)
                if t < steps - 1:
                    pgA = gpsum.tile([128, 512], F32, tag="pgA")
                    pgB = gpsum.tile([128, 512], F32, tag="pgB")
                    open_bias(pgA, pgB)
                hT = hTn

            # ---------- phase 5: select + linear + log_softmax ----------
            six = cst.tile([128, 1], I32, tag="six")
            nc.sync.dma_start(out=six[:], in_=selidx[:])
            hsel = cst.tile([128, HIDDEN], F16, tag="hsel")
            nc.gpsimd.indirect_dma_start(
                out=hsel[:], out_offset=None,
                in_=ring[:, :],
                in_offset=bass.IndirectOffsetOnAxis(ap=six[:, :1], axis=0))
            # transpose hsel[0:64] -> hselT chunks
            pt2 = tpsum.tile([128, NKC * 64], F16, tag="pt")
            for k in range(NKC):
                nc.tensor.transpose(out=pt2[:, k * 64:(k + 1) * 64],
                                    in_=hsel[0:64, k * 128:(k + 1) * 128],
                                    identity=ident[:])
            hselT = cst.tile([128, NKC * 64], F16, tag="hselT")
            nc.vector.tensor_copy(out=hselT[:], in_=pt2[:])
            # lin = hsel' @ Wo_eff.T + bo_eff
            wo_sb = wpool.tile([128, NKC * HIDDEN], F16, tag="wo")
            for k in range(NKC):
                nc.sync.dma_start(out=wo_sb[:, k * HIDDEN:(k + 1) * HIDDEN],
                                  in_=wo[k * 128:(k + 1) * 128, :])
            plA = gpsum.tile([64, 512], F32, tag="pgA")
            plB = gpsum.tile([64, 512], F32, tag="pgB")
            pls = (plA, plB)
            for k in range(NKC):
                for n in range(2):
                    nc.tensor.matmul(out=pls[n][:, :], lhsT=hselT[:, k * 64:(k + 1) * 64],
                                     rhs=wo_sb[:, k * HIDDEN + n * 512:k * HIDDEN + (n + 1) * 512],
                                     start=(k == 0), stop=False)
            for n in range(2):
                ns = slice(n * 512, (n + 1) * 512)
                nc.tensor.matmul(out=pls[n][:, :], lhsT=ones64[:], rhs=bo_sb[:, ns],
                                 start=False, stop=True)
            lin = cst.tile([64, HIDDEN], F16, tag="lin")
            nc.vector.tensor_copy(out=lin[:, 0:512], in_=plA[:])
            nc.vector.tensor_copy(out=lin[:, 512:1024], in_=plB[:])
            pt3 = tpsum.tile([128, NKC * 64], F16, tag="pt")
            for k in range(NKC):
                nc.tensor.transpose(out=pt3[:, k * 64:(k + 1) * 64],
                                    in_=lin[:, k * 128:(k + 1) * 128],
                                    identity=ident[:])
            linT = cst.tile([128, NKC * 64], F16, tag="linT")
            nc.vector.tensor_copy(out=linT[:], in_=pt3[:])
            wl_sb = cst.tile([128, NKC * NCLS], F16, tag="wl")
            for k in range(NKC):
                nc.sync.dma_start(out=wl_sb[:, k * NCLS:(k + 1) * NCLS],
                                  in_=wlin[k * 128:(k + 1) * 128, :])
            pz = upsum.tile([64, NCLS], F32, tag="pu")
            for k in range(NKC):
                nc.tensor.matmul(out=pz[:], lhsT=linT[:, k * 64:(k + 1) * 64],
                                 rhs=wl_sb[:, k * NCLS:(k + 1) * NCLS],
                                 start=(k == 0), stop=(k == NKC - 1))
            # log_softmax over the 2 classes (free axis)
            m = cst.tile([64, 1], F32, tag="m")
            nc.vector.tensor_reduce(out=m[:], in_=pz[:], axis=mybir.AxisListType.X,
                                    op=mybir.AluOpType.max)
            xm = cst.tile([64, NCLS], F32, tag="xm")
            nc.vector.tensor_scalar(out=xm[:], in0=pz[:], scalar1=m[:], scalar2=None,
                                    op0=mybir.AluOpType.subtract)
            esum = cst.tile([64, 1], F32, tag="esum")
            ex = cst.tile([64, NCLS], F32, tag="ex")
            nc.scalar.activation(ex[:], xm[:], mybir.ActivationFunctionType.Exp,
                                 accum_out=esum[:])
            lns = cst.tile([64, 1], F32, tag="lns")
            nc.scalar.activation(lns[:], esum[:], mybir.ActivationFunctionType.Ln)
            res = cst.tile([64, NCLS], F32, tag="res")
            nc.vector.tensor_scalar(out=res[:], in0=xm[:], scalar1=lns[:], scalar2=None,
                                    op0=mybir.AluOpType.subtract)
            nc.sync.dma_start(out=out_ext[:, :], in_=res[:])

    nc.compile()
    return nc


def _prep(x, lengths, emb, W_i, b_i, W_f, b_f, W_h, b_h, W_o, b_o, W_lin, b_lin,
          steps=S):
    f16 = np.float16
    f32 = np.float32
    # folded weights for the h' = 2h - 1 reformulation (see _build docstring):
    #   psum_f = h' @ (0.25 Wf).T + (0.5 bf + 0.25 rowsum Wf)   -> S' = tanh(psum_f)
    #   psum_h = h' @ (0.50 Wh).T + (bh + 0.5 rowsum Wh)        -> T  = tanh(psum_h)
    #   inp2   = relu(e @ (2 Wi).T + 2 bi)
    #   h'_new = S' + T * inp2
    #   lin    = h'_sel @ (0.5 Wo).T + (bo + 0.5 rowsum Wo)
    W_f = W_f.astype(f32); W_h = W_h.astype(f32); W_o = W_o.astype(f32)
    Wf_eff = 0.25 * W_f
    bf_eff = 0.5 * b_f.astype(f32) + 0.25 * W_f.sum(axis=1)
    Wh_eff = 0.5 * W_h
    bh_eff = b_h.astype(f32) + 0.5 * W_h.sum(axis=1)
    Wi_eff = 2.0 * W_i.astype(f32)
    bi_eff = 2.0 * b_i.astype(f32)
    Wo_eff = 0.5 * W_o
    bo_eff = b_o.astype(f32) + 0.5 * W_o.sum(axis=1)

    # contiguous per-vocab-tile layout: embt2[i*128+p, e*128+c] = emb[i*128+c, e*128+p]
    E = emb.astype(f16)
    embt2 = np.ascontiguousarray(
        E.reshape(NVT, 128, NEC, 128).transpose(0, 3, 2, 1).reshape(NVT * 128, EMBED))
    x_tm = np.ascontiguousarray(x.T)  # [S, B] t-major
    idx_tm = np.ascontiguousarray(x_tm.reshape(TOK // 128, 128).T).astype(np.int32)  # [128, 256] col-major
    sel = ((lengths.astype(np.int64) - 1) * B + np.arange(B)).astype(np.int32)
    selpad = np.zeros((128, 1), np.int32)
    selpad[:B, 0] = sel
    maps = []
    for c in range(NCORES):
        hsl = slice(c * HC, (c + 1) * HC)
        maps.append({
            "embt": embt2,
            "wi": np.ascontiguousarray(Wi_eff[hsl, :].T.astype(f16)),
            "bi": bi_eff[None, hsl].astype(f16),
            "wf": np.ascontiguousarray(Wf_eff.T.astype(f16)),
            "wh": np.ascontiguousarray(Wh_eff.T.astype(f16)),
            "bf_r": bf_eff[None, :].astype(f16),
            "bh_r": bh_eff[None, :].astype(f16),
            "wo": np.ascontiguousarray(Wo_eff.T.astype(f16)),
            "bo_r": bo_eff[None, :].astype(f16),
            "wlin": np.ascontiguousarray(W_lin.T.astype(f16)),
            "idx": idx_tm,
            "selidx": selpad,
        })
    return maps


def _run(inputs, steps=S, trace=False):
    key = steps
    if key not in _CACHE:
        _CACHE[key] = _build(steps)
    nc = _CACHE[key]
    maps = _prep(**inputs, steps=steps)
    res = run_bass_kernel_spmd(nc, maps, core_ids=list(range(NCORES)), trace=trace)
    return res


def kernel(**inputs) -> np.ndarray:
    res = _run(inputs, steps=S, trace=False)
    return res.results[0]["out"]


if __name__ == "__main__":
    steps = int(os.environ.get("KSTEPS", "8"))
    rng = np.random.default_rng(0)
    x = rng.integers(0, VOCAB, size=(B, S)).astype(np.int64)
    lengths = rng.integers(1, steps + 1, size=(B,)).astype(np.int64)
    lengths[0] = steps
    s_e, s_h = 1 / np.sqrt(EMBED), 1 / np.sqrt(HIDDEN)
    ins = dict(
        x=x, lengths=lengths,
        emb=rng.normal(size=(VOCAB, EMBED)).astype(np.float32),
        W_i=rng.uniform(-s_e, s_e, (HIDDEN, EMBED)).astype(np.float32),
        b_i=rng.uniform(-s_e, s_e, (HIDDEN,)).astype(np.float32),
        W_f=rng.uniform(-s_h, s_h, (HIDDEN, HIDDEN)).astype(np.float32),
        b_f=rng.uniform(-s_h, s_h, (HIDDEN,)).astype(np.float32),
        W_h=rng.uniform(-s_h, s_h, (HIDDEN, HIDDEN)).astype(np.float32),
        b_h=rng.uniform(-s_h, s_h, (HIDDEN,)).astype(np.float32),
        W_o=rng.uniform(-s_h, s_h, (HIDDEN, HIDDEN)).astype(np.float32),
        b_o=rng.uniform(-s_h, s_h, (HIDDEN,)).astype(np.float32),
        W_lin=rng.uniform(-s_h, s_h, (NCLS, HIDDEN)).astype(np.float32),
        b_lin=np.zeros((NCLS,), np.float32),
    )
    # numpy reference (on truncated steps)
    def npref(steps):
        e = ins["emb"][x]  # [B, S, E]
        h = np.zeros((B, HIDDEN), np.float32)
        outs = np.zeros((steps, B, HIDDEN), np.float32)
        for t in range(steps):
            et_ = e[:, t, :]
            inp = np.maximum(et_ @ ins["W_i"].T + ins["b_i"], 0)
            hf = 1 / (1 + np.exp(-(h @ ins["W_f"].T + ins["b_f"])))
            hh = np.tanh(h @ ins["W_h"].T + ins["b_h"])
            h = hf + hh * inp
            outs[t] = h
        li = outs[lengths - 1, np.arange(B)]
        lin = li @ ins["W_o"].T + ins["b_o"]
        lg = lin @ ins["W_lin"].T + ins["b_lin"]
        lg = lg - lg.max(1, keepdims=True)
        return lg - np.log(np.exp(lg).sum(1, keepdims=True))

    expected = npref(steps)
    res = _run(ins, steps=steps, trace=False)
    got = res.results[0]["out"]
    err = np.linalg.norm(got - expected) / np.linalg.norm(expected)
    print("expected[:3]:", expected[:3])
    print("got[:3]:", got[:3])
    print("rel_err:", err)


# revision 10
# speedup vs baseline: 2.9777x; 1.7932x over previous
"""Trainium2 Bass kernel for nn_FCLSTM: embedding -> custom LSTM-ish recurrence -> select -> linear -> log_softmax.

Self-contained: hardcodes shapes. kernel(**inputs) takes full numpy inputs, returns [64, 2] fp32.

v2 redesign vs baseline:
  - State transform h' = 2h - 1 turns sigmoid(f)+tanh(g)*inp into tanh(f')+tanh(g')*inp2
    with all scales/biases folded into the weights host-side -> ONE tanh ACT op per
    psum half ([128,512]) instead of sigmoid+tanh pairs.
  - Bias matmuls issued as next-step PSUM group openers (fill the PE pipeline bubble).
  - h_new -> hT transposes moved off the PE onto the DMA XBAR transpose engine.
  - Per-chunk hT tiles so next-step matmuls wait only on their own chunk.
  - Contiguous embedding-table layout (one [128,512] DMA per vocab tile).
"""
import os
import numpy as np

import concourse.bacc as bacc
import concourse.bass as bass
import concourse.mybir as mybir
from concourse import library_config  # noqa: F401
from concourse.tile import TileContext
from concourse.masks import make_identity
from concourse.bass_utils import run_bass_kernel_spmd

VOCAB, EMBED, HIDDEN, NCLS = 32000, 512, 1024, 2
B, S = 64, 512
NCORES = 8
HC = HIDDEN // NCORES          # 128 per-core H slice for the U table
NVT = VOCAB // 128             # 250 vocab tiles
NEC = EMBED // 128             # 4 embed (contraction) chunks
NKC = HIDDEN // 128            # 8 hidden contraction chunks
TCH = S // 8                   # 64 steps per AllGather time-chunk
TOK = B * S                    # 32768 tokens
F16 = mybir.dt.float16
F32 = mybir.dt.float32
I32 = mybir.dt.int32
Tanh = mybir.ActivationFunctionType.Tanh

# Plan A: one tanh per [128,512] psum half; DVE mul reads T at partition base 64.
# Plan B fallback (DVE_SHIFT=False): two tanh ACT ops per half, both landing at base 0
# (partition-shifted ACT reads, baseline-proven).
DVE_SHIFT = False

_CACHE = {}


def _build(steps=S):
    nc = bacc.Bacc("TRN2", target_bir_lowering=False, debug=False, num_devices=NCORES)

    # ---------- inputs ----------
    embt = nc.dram_tensor("embt", [NVT * 128, EMBED], F16, kind="ExternalInput")
    wi = nc.dram_tensor("wi", [EMBED, HC], F16, kind="ExternalInput")
    bi = nc.dram_tensor("bi", [1, HC], F16, kind="ExternalInput")
    wf = nc.dram_tensor("wf", [HIDDEN, HIDDEN], F16, kind="ExternalInput")
    wh = nc.dram_tensor("wh", [HIDDEN, HIDDEN], F16, kind="ExternalInput")
    bf_r = nc.dram_tensor("bf_r", [1, HIDDEN], F16, kind="ExternalInput")
    bh_r = nc.dram_tensor("bh_r", [1, HIDDEN], F16, kind="ExternalInput")
    wo = nc.dram_tensor("wo", [HIDDEN, HIDDEN], F16, kind="ExternalInput")
    bo_r = nc.dram_tensor("bo_r", [1, HIDDEN], F16, kind="ExternalInput")
    wlin = nc.dram_tensor("wlin", [HIDDEN, NCLS], F16, kind="ExternalInput")
    idx = nc.dram_tensor("idx", [128, TOK // 128], I32, kind="ExternalInput")
    ident2 = nc.dram_tensor("ident2", [128, 64], F16, kind="ExternalInput")
    selidx = nc.dram_tensor("selidx", [128, 1], I32, kind="ExternalInput")
    out_ext = nc.dram_tensor("out", [B, NCLS], F32, kind="ExternalOutput")

    ntch = (steps + TCH - 1) // TCH  # number of time chunks actually used

    with TileContext(nc) as tc:
        with (
            tc.tile_pool(name="dram", bufs=1, space="DRAM") as dram,
            tc.tile_pool(name="const", bufs=1) as cst,
            tc.tile_pool(name="w", bufs=1) as wpool,
            tc.tile_pool(name="uph", bufs=4) as uph,
            tc.tile_pool(name="upsum", bufs=2, space="PSUM") as upsum,
            tc.tile_pool(name="rec", bufs=2) as rec,
            tc.tile_pool(name="inp", bufs=3) as inpool,
            tc.tile_pool(name="gpsum", bufs=2, space="PSUM") as gpsum,
            tc.tile_pool(name="tpsum", bufs=1, space="PSUM") as tpsum,
        ):
            # ---------- DRAM scratch ----------
            u_dram = dram.tile([VOCAB, HC], F16)
            agin = [dram.tile([B * TCH, HC], F16, name=f"agin{j}") for j in range(ntch)]
            gath = [dram.tile([NCORES * B * TCH, HC], F16, name=f"gath{j}", addr_space="Shared") for j in range(ntch)]
            ring = dram.tile([TOK, HIDDEN], F16)

            # ---------- constants / weights to SBUF ----------
            ones64 = cst.tile([1, 64], F16, tag="ones64")
            nc.vector.memset(ones64[:], 1.0)
            ones128 = cst.tile([1, 128], F16, tag="ones128")
            nc.vector.memset(ones128[:], 1.0)
            ident = cst.tile([64, 64], F16, tag="ident")
            make_identity(nc, ident[:])
            id2_sb = cst.tile([128, 64], F16, tag="id2")
            nc.sync.dma_start(out=id2_sb[:], in_=ident2[:, :])

            wi_sb = cst.tile([128, NEC * HC], F16, tag="wi")
            for e in range(NEC):
                nc.sync.dma_start(out=wi_sb[:, e * HC:(e + 1) * HC],
                                  in_=wi[e * 128:(e + 1) * 128, :])
            bi_sb = cst.tile([1, HC], F16, tag="bi")
            nc.sync.dma_start(out=bi_sb[:], in_=bi[:])
            bf_sb = cst.tile([1, HIDDEN], F16, tag="bf")
            nc.sync.dma_start(out=bf_sb[:], in_=bf_r[:])
            bh_sb = cst.tile([1, HIDDEN], F16, tag="bh")
            nc.sync.dma_start(out=bh_sb[:], in_=bh_r[:])
            bo_sb = cst.tile([1, HIDDEN], F16, tag="bo")
            nc.sync.dma_start(out=bo_sb[:], in_=bo_r[:])

            wf_sb = wpool.tile([128, NKC * HIDDEN], F16, tag="wf")
            wh_sb = wpool.tile([128, NKC * HIDDEN], F16, tag="wh")
            for k in range(NKC):
                nc.sync.dma_start(out=wf_sb[:, k * HIDDEN:(k + 1) * HIDDEN],
                                  in_=wf[k * 128:(k + 1) * 128, :])
                nc.sync.dma_start(out=wh_sb[:, k * HIDDEN:(k + 1) * HIDDEN],
                                  in_=wh[k * 128:(k + 1) * 128, :])

            # ---------- phase 1: U table  U_c = relu(emb @ (2 WiT_c) + 2 bi_c) ----------
            for i in range(NVT):
                et = uph.tile([128, EMBED], F16, tag="et")
                nc.sync.dma_start(out=et[:], in_=embt[i * 128:(i + 1) * 128, :])
                pu = upsum.tile([128, HC], F32, tag="pu")
                for e in range(NEC):
                    nc.tensor.matmul(out=pu[:], lhsT=et[:, e * 128:(e + 1) * 128],
                                     rhs=wi_sb[:, e * HC:(e + 1) * HC],
                                     start=(e == 0), stop=False)
                nc.tensor.matmul(out=pu[:], lhsT=ones128[:], rhs=bi_sb[:],
                                 start=False, stop=True)
                u_sb = uph.tile([128, HC], F16, tag="usb")
                nc.scalar.activation(u_sb[:], pu[:], mybir.ActivationFunctionType.Relu)
                nc.scalar.dma_start(out=u_dram[i * 128:(i + 1) * 128, :], in_=u_sb[:])

            # ---------- phase 2: gather inp_c rows (t-major) + phase 3: AllGather ----------
            ng_per_ch = (B * TCH) // 128  # 32 gather calls per time chunk
            ncalls = ntch * ng_per_ch
            idx_all = cst.tile([128, 256], I32, tag="idx_all")
            nc.sync.dma_start(out=idx_all[:, :ncalls], in_=idx[:, 0:ncalls])
            for j in range(ntch):
                for g in range(ng_per_ch):
                    k = j * ng_per_ch + g
                    gt = uph.tile([128, HC], F16, tag="gt")
                    nc.gpsimd.indirect_dma_start(
                        out=gt[:], out_offset=None,
                        in_=u_dram[:, :],
                        in_offset=bass.IndirectOffsetOnAxis(ap=idx_all[:, k:k + 1], axis=0))
                    nc.sync.dma_start(out=agin[j][g * 128:(g + 1) * 128, :], in_=gt[:])
                nc.gpsimd.collective_compute(
                    "AllGather", mybir.AluOpType.bypass,
                    replica_groups=[list(range(NCORES))],
                    ins=[agin[j].opt()], outs=[gath[j].opt()])

            # ---------- phase 4: recurrence in h' = 2h - 1 space ----------
            # psum halves: rows 0:64 = f-gate (-> S' = tanh), rows 64:128 = h-gate (-> T = tanh)
            # h'_new = S' + T * inp2
            # Steps are processed in PAIRS sharing one inp2 tile: even step t=2m computes
            # its tail on partitions 0:64, odd step on partitions 64:128 (ACT does the
            # partition shift; DVE ops stay base-aligned). inp2 pair tiles are loaded with
            # 8 contiguous [128,128] DMAs (one per core-slice of the gathered U table).
            hT = []
            for g in range(2):
                t0 = rec.tile([128, 256], F16, tag=f"hTg{g}")
                nc.vector.memset(t0[:], -1.0)
                hT.append(t0)

            def open_bias(pgA, pgB):
                for half, pgX in ((0, pgA), (1, pgB)):
                    ns = slice(half * 512, (half + 1) * 512)
                    nc.tensor.matmul(out=pgX[0:64, :], lhsT=ones64[:], rhs=bf_sb[:, ns],
                                     start=True, stop=False, tile_position=(0, 0))
                    nc.tensor.matmul(out=pgX[64:128, :], lhsT=ones64[:], rhs=bh_sb[:, ns],
                                     start=True, stop=False, tile_position=(0, 64))

            pgA = gpsum.tile([128, 512], F32, tag="pgA")
            pgB = gpsum.tile([128, 512], F32, tag="pgB")
            open_bias(pgA, pgB)

            inp2 = None
            hnp = [None, None]
            for t in range(steps):
                j, tl = t // TCH, t % TCH
                par = t % 2          # 0: rows 0:64, 1: rows 64:128
                rs = slice(64 * par, 64 * par + 64)
                if par == 0:
                    # load inp2 for this step pair: 8 contiguous [128,128] reads
                    nrow = 128 if t + 1 < steps else 64
                    inp2 = inpool.tile([128, HIDDEN], F16, tag="inp", name="inp2")
                    for c in range(NCORES):
                        src = bass.AP(tensor=gath[j].tensor,
                                      offset=c * B * TCH * HC + tl * B * HC,
                                      ap=[[HC, nrow], [1, HC]])
                        eng = nc.sync if c % 2 == 0 else nc.gpsimd
                        eng.dma_start(out=inp2[0:nrow, c * HC:(c + 1) * HC], in_=src)
                    hnp = [rec.tile([128, 512], F16, tag=f"hnp{h}", name=f"hnp{h}") for h in (0, 1)]

                th = [None, None]
                hTn = [None, None]
                pgs = (pgA, pgB)
                for half in (0, 1):
                    pg = pgs[half]
                    for k in range(NKC):
                        woff = k * HIDDEN + half * 512
                        lhs = hT[k // 4][:, (k % 4) * 64:(k % 4 + 1) * 64]
                        nc.tensor.matmul(out=pg[0:64, :], lhsT=lhs,
                                         rhs=wf_sb[:, woff:woff + 512],
                                         start=False, stop=(k == NKC - 1),
                                         tile_position=(0, 0))
                        nc.tensor.matmul(out=pg[64:128, :], lhsT=lhs,
                                         rhs=wh_sb[:, woff:woff + 512],
                                         start=False, stop=(k == NKC - 1),
                                         tile_position=(0, 64))
                    # T at cols 0:512, S' at cols 512:1024, on this parity's partition rows
                    th[half] = rec.tile([128, 1024], F16, tag=f"th{half}", name=f"th{half}")
                    nc.scalar.activation(th[half][rs, 0:512], pg[64:128, :], Tanh)
                    nc.scalar.activation(th[half][rs, 512:1024], pg[0:64, :], Tanh)
                    # combine on this parity's rows (all operands base-aligned)
                    hs = slice(half * 512, (half + 1) * 512)
                    nc.vector.tensor_mul(out=hnp[half][rs, :], in0=th[half][rs, 0:512],
                                         in1=inp2[rs, hs])
                    nc.vector.tensor_add(out=hnp[half][rs, :], in0=hnp[half][rs, :],
                                         in1=th[half][rs, 512:1024])
                # open next step's psum with the bias rows (fills PE bubble, keeps HAM warm)
                if t < steps - 1:
                    pgA = gpsum.tile([128, 512], F32, tag="pgA")
                    pgB = gpsum.tile([128, 512], F32, tag="pgB")
                    open_bias(pgA, pgB)
                # hT chunk-groups for the next step: PE transposes (PE idles here anyway)
                idp = id2_sb[64 * par:64 * par + 64, :]
                pt = tpsum.tile([128, 512], F16, tag="pt")
                for half in (0, 1):
                    for ki in range(4):
                        kk = half * 4 + ki
                        nc.tensor.transpose(out=pt[:, kk * 64:(kk + 1) * 64],
                                            in_=hnp[half][rs, ki * 128:(ki + 1) * 128],
                                            identity=idp)
                    hTn[half] = rec.tile([128, 256], F16, tag=f"hTg{half}", name=f"hTn{half}")
                    nc.vector.tensor_copy(out=hTn[half][:, :],
                                          in_=pt[:, half * 256:(half + 1) * 256])
                if par == 1 or t == steps - 1:
                    # one ring write per step pair per half ([t*B-64 .. t*B+64) rows)
                    nr = 128 if par == 1 else 64
                    r0 = (t - par) * B
                    for half in (0, 1):
                        nc.scalar.dma_start(
                            out=ring[r0:r0 + nr, half * 512:(half + 1) * 512],
                            in_=hnp[half][0:nr, :])
                hT = hTn

            # ---------- phase 5: select + linear + log_softmax ----------
            six = cst.tile([128, 1], I32, tag="six")
            nc.sync.dma_start(out=six[:], in_=selidx[:])
            hsel = cst.tile([128, HIDDEN], F16, tag="hsel")
            nc.gpsimd.indirect_dma_start(
                out=hsel[:], out_offset=None,
                in_=ring[:, :],
                in_offset=bass.IndirectOffsetOnAxis(ap=six[:, :1], axis=0))
            # transpose hsel[0:64] -> hselT chunks
            pt2 = tpsum.tile([128, NKC * 64], F16, tag="pt")
            for k in range(NKC):
                nc.tensor.transpose(out=pt2[:, k * 64:(k + 1) * 64],
                                    in_=hsel[0:64, k * 128:(k + 1) * 128],
                                    identity=ident[:])
            hselT = cst.tile([128, NKC * 64], F16, tag="hselT")
            nc.vector.tensor_copy(out=hselT[:], in_=pt2[:])
            # lin = hsel' @ Wo_eff.T + bo_eff
            wo_sb = wpool.tile([128, NKC * HIDDEN], F16, tag="wo")
            for k in range(NKC):
                nc.sync.dma_start(out=wo_sb[:, k * HIDDEN:(k + 1) * HIDDEN],
                                  in_=wo[k * 128:(k + 1) * 128, :])
            plA = gpsum.tile([64, 512], F32, tag="pgA")
            plB = gpsum.tile([64, 512], F32, tag="pgB")
            pls = (plA, plB)
            for k in range(NKC):
                for n in range(2):
                    nc.tensor.matmul(out=pls[n][:, :], lhsT=hselT[:, k * 64:(k + 1) * 64],
                                     rhs=wo_sb[:, k * HIDDEN + n * 512:k * HIDDEN + (n + 1) * 512],
                                     start=(k == 0), stop=False)
            for n in range(2):
                ns = slice(n * 512, (n + 1) * 512)
                nc.tensor.matmul(out=pls[n][:, :], lhsT=ones64[:], rhs=bo_sb[:, ns],
                                 start=False, stop=True)
            lin = cst.tile([64, HIDDEN], F16, tag="lin")
            nc.vector.tensor_copy(out=lin[:, 0:512], in_=plA[:])
            nc.vector.tensor_copy(out=lin[:, 512:1024], in_=plB[:])
            pt3 = tpsum.tile([128, NKC * 64], F16, tag="pt")
            for k in range(NKC):
                nc.tensor.transpose(out=pt3[:, k * 64:(k + 1) * 64],
                                    in_=lin[:, k * 128:(k + 1) * 128],
                                    identity=ident[:])
            linT = cst.tile([128, NKC * 64], F16, tag="linT")
            nc.vector.tensor_copy(out=linT[:], in_=pt3[:])
            wl_sb = cst.tile([128, NKC * NCLS], F16, tag="wl")
            for k in range(NKC):
                nc.sync.dma_start(out=wl_sb[:, k * NCLS:(k + 1) * NCLS],
                                  in_=wlin[k * 128:(k + 1) * 128, :])
            pz = upsum.tile([64, NCLS], F32, tag="pu")
            for k in range(NKC):
                nc.tensor.matmul(out=pz[:], lhsT=linT[:, k * 64:(k + 1) * 64],
                                 rhs=wl_sb[:, k * NCLS:(k + 1) * NCLS],
                                 start=(k == 0), stop=(k == NKC - 1))
            # log_softmax over the 2 classes (free axis)
            m = cst.tile([64, 1], F32, tag="m")
            nc.vector.tensor_reduce(out=m[:], in_=pz[:], axis=mybir.AxisListType.X,
                                    op=mybir.AluOpType.max)
            xm = cst.tile([64, NCLS], F32, tag="xm")
            nc.vector.tensor_scalar(out=xm[:], in0=pz[:], scalar1=m[:], scalar2=None,
                                    op0=mybir.AluOpType.subtract)
            esum = cst.tile([64, 1], F32, tag="esum")
            ex = cst.tile([64, NCLS], F32, tag="ex")
            nc.scalar.activation(ex[:], xm[:], mybir.ActivationFunctionType.Exp,
                                 accum_out=esum[:])
            lns = cst.tile([64, 1], F32, tag="lns")
            nc.scalar.activation(lns[:], esum[:], mybir.ActivationFunctionType.Ln)
            res = cst.tile([64, NCLS], F32, tag="res")
            nc.vector.tensor_scalar(out=res[:], in0=xm[:], scalar1=lns[:], scalar2=None,
                                    op0=mybir.AluOpType.subtract)
            nc.sync.dma_start(out=out_ext[:, :], in_=res[:])

    nc.compile()
    return nc


def _prep(x, lengths, emb, W_i, b_i, W_f, b_f, W_h, b_h, W_o, b_o, W_lin, b_lin,
          steps=S):
    f16 = np.float16
    f32 = np.float32
    # folded weights for the h' = 2h - 1 reformulation (see _build docstring):
    #   psum_f = h' @ (0.25 Wf).T + (0.5 bf + 0.25 rowsum Wf)   -> S' = tanh(psum_f)
    #   psum_h = h' @ (0.50 Wh).T + (bh + 0.5 rowsum Wh)        -> T  = tanh(psum_h)
    #   inp2   = relu(e @ (2 Wi).T + 2 bi)
    #   h'_new = S' + T * inp2
    #   lin    = h'_sel @ (0.5 Wo).T + (bo + 0.5 rowsum Wo)
    W_f = W_f.astype(f32); W_h = W_h.astype(f32); W_o = W_o.astype(f32)
    Wf_eff = 0.25 * W_f
    bf_eff = 0.5 * b_f.astype(f32) + 0.25 * W_f.sum(axis=1)
    Wh_eff = 0.5 * W_h
    bh_eff = b_h.astype(f32) + 0.5 * W_h.sum(axis=1)
    Wi_eff = 2.0 * W_i.astype(f32)
    bi_eff = 2.0 * b_i.astype(f32)
    Wo_eff = 0.5 * W_o
    bo_eff = b_o.astype(f32) + 0.5 * W_o.sum(axis=1)

    # contiguous per-vocab-tile layout: embt2[i*128+p, e*128+c] = emb[i*128+c, e*128+p]
    E = emb.astype(f16)
    embt2 = np.ascontiguousarray(
        E.reshape(NVT, 128, NEC, 128).transpose(0, 3, 2, 1).reshape(NVT * 128, EMBED))
    x_tm = np.ascontiguousarray(x.T)  # [S, B] t-major
    idx_tm = np.ascontiguousarray(x_tm.reshape(TOK // 128, 128).T).astype(np.int32)  # [128, 256] col-major
    sel = ((lengths.astype(np.int64) - 1) * B + np.arange(B)).astype(np.int32)
    selpad = np.zeros((128, 1), np.int32)
    selpad[:B, 0] = sel
    id2 = np.concatenate([np.eye(64, dtype=np.float16), np.eye(64, dtype=np.float16)], axis=0)
    maps = []
    for c in range(NCORES):
        hsl = slice(c * HC, (c + 1) * HC)
        maps.append({
            "embt": embt2,
            "wi": np.ascontiguousarray(Wi_eff[hsl, :].T.astype(f16)),
            "bi": bi_eff[None, hsl].astype(f16),
            "wf": np.ascontiguousarray(Wf_eff.T.astype(f16)),
            "wh": np.ascontiguousarray(Wh_eff.T.astype(f16)),
            "bf_r": bf_eff[None, :].astype(f16),
            "bh_r": bh_eff[None, :].astype(f16),
            "wo": np.ascontiguousarray(Wo_eff.T.astype(f16)),
            "bo_r": bo_eff[None, :].astype(f16),
            "wlin": np.ascontiguousarray(W_lin.T.astype(f16)),
            "idx": idx_tm,
            "selidx": selpad,
            "ident2": id2,
        })
    return maps


def _run(inputs, steps=S, trace=False):
    key = steps
    if key not in _CACHE:
        _CACHE[key] = _build(steps)
    nc = _CACHE[key]
    maps = _prep(**inputs, steps=steps)
    res = run_bass_kernel_spmd(nc, maps, core_ids=list(range(NCORES)), trace=trace)
    return res


def kernel(**inputs) -> np.ndarray:
    res = _run(inputs, steps=S, trace=False)
    return res.results[0]["out"]


if __name__ == "__main__":
    steps = int(os.environ.get("KSTEPS", "8"))
    rng = np.random.default_rng(0)
    x = rng.integers(0, VOCAB, size=(B, S)).astype(np.int64)
    lengths = rng.integers(1, steps + 1, size=(B,)).astype(np.int64)
    lengths[0] = steps
    s_e, s_h = 1 / np.sqrt(EMBED), 1 / np.sqrt(HIDDEN)
    ins = dict(
        x=x, lengths=lengths,
        emb=rng.normal(size=(VOCAB, EMBED)).astype(np.float32),
        W_i=rng.uniform(-s_e, s_e, (HIDDEN, EMBED)).astype(np.float32),
        b_i=rng.uniform(-s_e, s_e, (HIDDEN,)).astype(np.float32),
        W_f=rng.uniform(-s_h, s_h, (HIDDEN, HIDDEN)).astype(np.float32),
        b_f=rng.uniform(-s_h, s_h, (HIDDEN,)).astype(np.float32),
        W_h=rng.uniform(-s_h, s_h, (HIDDEN, HIDDEN)).astype(np.float32),
        b_h=rng.uniform(-s_h, s_h, (HIDDEN,)).astype(np.float32),
        W_o=rng.uniform(-s_h, s_h, (HIDDEN, HIDDEN)).astype(np.float32),
        b_o=rng.uniform(-s_h, s_h, (HIDDEN,)).astype(np.float32),
        W_lin=rng.uniform(-s_h, s_h, (NCLS, HIDDEN)).astype(np.float32),
        b_lin=np.zeros((NCLS,), np.float32),
    )
    # numpy reference (on truncated steps)
    def npref(steps):
        e = ins["emb"][x]  # [B, S, E]
        h = np.zeros((B, HIDDEN), np.float32)
        outs = np.zeros((steps, B, HIDDEN), np.float32)
        for t in range(steps):
            et_ = e[:, t, :]
            inp = np.maximum(et_ @ ins["W_i"].T + ins["b_i"], 0)
            hf = 1 / (1 + np.exp(-(h @ ins["W_f"].T + ins["b_f"])))
            hh = np.tanh(h @ ins["W_h"].T + ins["b_h"])
            h = hf + hh * inp
            outs[t] = h
        li = outs[lengths - 1, np.arange(B)]
        lin = li @ ins["W_o"].T + ins["b_o"]
        lg = lin @ ins["W_lin"].T + ins["b_lin"]
        lg = lg - lg.max(1, keepdims=True)
        return lg - np.log(np.exp(lg).sum(1, keepdims=True))

    expected = npref(steps)
    res = _run(ins, steps=steps, trace=False)
    got = res.results[0]["out"]
    err = np.linalg.norm(got - expected) / np.linalg.norm(expected)
    print("expected[:3]:", expected[:3])
    print("got[:3]:", got[:3])
    print("rel_err:", err)
